# revision 22
# baseline (speedup 1.0000x reference)
"""Talking-heads attention (b=4, n=2048, d=512, h=8) on 8 TRN2 NeuronCores.

Strategy
--------
dots2_g = sum_h mix_pre[h,g] (q_h . k_h) has the head-sum inside a 512-wide
contraction, so the dots matmuls contract (h,d)=512:

  attT_g[j,i] = sum_{hd} kT[hd,j] * (mix_pre[h,g] * qT[hd,i])

The g-dependence is only a per-64-row-block SCALE of qT, applied on the
Vector engine (qmix_g), so the per-g expanded-weight projections of the
old scheme disappear: one plain q/k projection (fp16) serves all heads,
and the dots stationary (kT) is head-independent.  mix_post + Wv + Wout +
gamma fold into expanded weights U (wu):

  B_g   = attn_g @ z          (z = normalized x, token-major, bf16)
  final = sum_g (B_g / rowsum_g) @ Wu_g + bout'

Rowsums run OFF the PE: exp tiles are accumulated over key-tiles on the
Vector engine and one f32r ones-matmul per (g, query-chunk) does the
partition reduction.  1/rowsum folds into the B PSUM evacuation.

Sharding: core c handles batch c//2 and query-row half c%2 (1024 rows) — no
collectives.  Each core's batch rows are permuted host-side so its query
rows come first (attention is invariant to key order).  Softmax runs
without max-subtraction (logits bounded ~25; exp fits bf16).
"""

import os
import sys

import numpy as np

for _p in ("/opt/trn_rl_repo", "/root/.axon_site/_ro/trn_rl_repo"):
    if os.path.isdir(_p) and _p not in sys.path:
        sys.path.append(_p)

import concourse.bacc as bacc
import concourse.tile as tile
from concourse import mybir
from concourse import masks
from concourse.bass_utils import run_bass_kernel_spmd

F32 = mybir.dt.float32
F32R = mybir.dt.float32r
BF16 = mybir.dt.bfloat16
FP16 = mybir.dt.float16

B, N, D = 4, 2048, 512
H, DH = 8, 64
NQ = N // 2            # query rows per core
N_CORES = 8
EPS = 1e-5
CT = D // 128           # 4 contraction tiles
NT = N // 128           # 16 token tiles (full sequence)
QT = NQ // 128          # 8 query tiles per core
NCH = NQ // 512         # 2 query chunks of 512


def build_program(attn_dtype=BF16):
    """Build + compile the SPMD single-core program. Returns nc."""
    nc = bacc.Bacc(
        "TRN2",
        target_bir_lowering=False,
        debug=False,
        enable_asserts=True,
        num_devices=1,
    )

    xb = nc.dram_tensor("xb", [N, D], F32, kind="ExternalInput").ap()
    wqk = nc.dram_tensor("wqk", [CT, 128, 2 * D], FP16, kind="ExternalInput").ap()
    bqk = nc.dram_tensor("bqk", [8, 128], F32, kind="ExternalInput").ap()
    mixv = nc.dram_tensor("mixv", [CT, 128, H], F32, kind="ExternalInput").ap()
    wu = nc.dram_tensor("wu", [CT, 128, H * D], BF16, kind="ExternalInput").ap()
    bo = nc.dram_tensor("bo", [1, D], F32, kind="ExternalInput").ap()
    out = nc.dram_tensor("out", [NQ, D], F32, kind="ExternalOutput").ap()

    import concourse.bass as bass

    with tile.TileContext(nc) as tc:
        with (
            tc.tile_pool(name="const", bufs=1) as const,
            tc.tile_pool(name="persist", bufs=1) as persist,
            tc.tile_pool(name="gw", bufs=2) as gw,
            tc.tile_pool(name="qm", bufs=2) as qm,
            tc.tile_pool(name="ln", bufs=6) as ln,
            tc.tile_pool(name="attw", bufs=12) as attw_pool,
            tc.tile_pool(name="rsp", bufs=2) as rsp,
            tc.tile_pool(name="btp", bufs=2) as btp,
            tc.tile_pool(name="rin", bufs=2) as rin,
            tc.tile_pool(name="ps_pd", bufs=2, space="PSUM") as ps_pd,
            tc.tile_pool(name="ps_pb", bufs=4, space="PSUM") as ps_pb,
            tc.tile_pool(name="ps_ms", bufs=2, space="PSUM") as ps_ms,
        ):
            ident_f = const.tile([128, 128], F32)
            masks.make_identity(nc, ident_f)
            ones_f = const.tile([128, 128], F32)
            nc.vector.memset(ones_f, 1.0)
            ones_r = const.tile([128, 128], F32R)
            nc.vector.tensor_copy(ones_r, ones_f)
            warm_b = const.tile([128, 256], BF16)
            nc.vector.memset(warm_b, 0.0)
            warm_s = const.tile([128, 128], BF16)
            nc.vector.memset(warm_s, 0.0)

            def warm(n):
                # keep the PE HAM clock-gate warm through DVE-bound stretches
                for _ in range(n):
                    pw = ps_pd.tile([128, 256], F32, tag="pd", name="pw")
                    nc.tensor.matmul(pw, warm_s, warm_b, start=True, stop=True)
            eps_t = const.tile([128, 1], F32)
            nc.vector.memset(eps_t, EPS)

            wqk_sb = const.tile([128, CT, 2 * D], FP16)
            bqk_sb = const.tile([128, 8], F32)
            mixv_sb = const.tile([128, CT, H], F32)
            bo_sb = const.tile([128, D], F32)

            zT = persist.tile([128, CT, N], FP16)     # normalized x, feature-major
            zt_b = persist.tile([128, NT, D], BF16)   # normalized x, token-major bf16
            qT = persist.tile([128, CT, NQ], FP16)    # q^T (scale folded), head-major
            kT = persist.tile([128, CT, N], FP16)     # k^T, head-major
            acc = persist.tile([128, QT, D], F32)     # final accumulator

            # ---- helpers ----
            def emit_proj(ic, qside):
                for co in (range(0, 4) if qside else range(4, 8)):
                    p = ps_ms.tile([128, 512], F32, tag="ms", name="pj")
                    for ct in range(CT):
                        nc.tensor.matmul(
                            p,
                            wqk_sb[:, ct, co * 128:(co + 1) * 128],
                            zT[:, ct, ic * 512:(ic + 1) * 512],
                            start=(ct == 0), stop=(ct == CT - 1),
                        )
                    dst = (qT[:, co, ic * 512:(ic + 1) * 512] if qside
                           else kT[:, co - 4, ic * 512:(ic + 1) * 512])
                    nc.scalar.activation(out=dst, in_=p,
                                         func=mybir.ActivationFunctionType.Identity,
                                         bias=bqk_sb[:, co:co + 1], scale=1.0)

            def emit_qmix(g, ch):
                # qmix = mix_pre[h,g] * qT chunk (per-64-row-block scale, DVE)
                qmix = qm.tile([128, CT, 512], FP16, tag="qx", name="qmix")
                for ct in range(CT):
                    nc.vector.tensor_scalar_mul(
                        qmix[:, ct, :], qT[:, ct, ch * 512:(ch + 1) * 512],
                        mixv_sb[:, ct, g:g + 1])
                return qmix

            def open_block():
                pbs = [ps_pb.tile([128, 512], F32, tag="pb", name=f"pb{cb}")
                       for cb in range(CT)]
                rs_acc = rsp.tile([128, 512], F32R, tag="rs")
                return pbs, rs_acc

            def attn_step(qmix, pbs, rs_acc, jt):
                pd = ps_pd.tile([128, 512], F32, tag="pd")
                for ct in range(CT):
                    nc.tensor.matmul(
                        pd, kT[:, ct, jt * 128:(jt + 1) * 128], qmix[:, ct, :],
                        start=(ct == 0), stop=(ct == CT - 1))
                aw = attw_pool.tile([128, 512], attn_dtype, tag="aw")
                nc.scalar.activation(out=aw, in_=pd,
                                     func=mybir.ActivationFunctionType.Exp)
                if jt == 0:
                    nc.vector.tensor_copy(rs_acc, aw)
                else:
                    nc.vector.tensor_add(out=rs_acc, in0=rs_acc, in1=aw)
                for cb in range(CT):
                    nc.tensor.matmul(pbs[cb], zt_b[:, jt, cb * 128:(cb + 1) * 128],
                                     aw, start=(jt == 0), stop=(jt == NT - 1))

            def close_block(g, ch, pbs, rs_acc, wu_g):
                # partition-reduce rowsums with one f32r matmul; 1/r folds into
                # the B evacuation; bout folds into the g==0 acc init
                prs = ps_ms.tile([128, 512], F32, tag="ms", name="prs")
                nc.tensor.matmul(prs, ones_r, rs_acc, start=True, stop=True)
                rinv = rin.tile([128, 512], F32, tag="ri")
                nc.vector.reciprocal_approx_fast(rinv, prs)
                bt = btp.tile([128, CT, 512], attn_dtype, tag="bt")
                for cb in range(CT):
                    nc.vector.tensor_mul(out=bt[:, cb, :], in0=pbs[cb], in1=rinv)
                for io in range(4):
                    it = ch * 4 + io
                    pf = ps_ms.tile([128, D], F32, tag="ms", name="pf")
                    for cb in range(CT):
                        nc.tensor.matmul(pf, bt[:, cb, io * 128:(io + 1) * 128],
                                         wu_g[:, cb, :],
                                         start=(cb == 0), stop=(cb == CT - 1))
                    nc.vector.tensor_add(out=acc[:, it, :], in0=pf,
                                         in1=(bo_sb if g == 0 else acc[:, it, :]))

            # ---- Phase 1+2 fused with the first attention block (g=0,ch=0):
            # LayerNorm tile t feeds transposes; projections are emitted as
            # their zT chunks complete; block0 attention steps trail t by 4.
            warm(16)
            xts = []
            for t in range(4):
                xt = ln.tile([128, D], F32, tag="xt", name=f"xt{t}")
                nc.sync.dma_start(out=xt, in_=xb[t * 128:(t + 1) * 128, :])
                xts.append(xt)
            nc.sync.dma_start(out=wqk_sb, in_=wqk.rearrange("t p c -> p t c"))
            nc.sync.dma_start(out=bqk_sb, in_=bqk.rearrange("t p -> p t"))
            nc.sync.dma_start(out=mixv_sb, in_=mixv.rearrange("t p c -> p t c"))
            nc.sync.dma_start(
                out=bo_sb,
                in_=bass.AP(tensor=bo.tensor, offset=bo.offset,
                            ap=[[0, 128], bo.ap[1]]),
            )
            wu0 = gw.tile([128, CT, D], BF16, tag="wu", name="wu0")
            nc.sync.dma_start(out=wu0, in_=wu[:, :, 0:D].rearrange("t p c -> p t c"))

            qmix0 = pbs0 = rs0 = None
            for t in range(NT):
                if t < 4:
                    xt = xts[t]
                else:
                    xt = ln.tile([128, D], F32, tag="xt")
                    nc.sync.dma_start(out=xt, in_=xb[t * 128:(t + 1) * 128, :])
                stats = ln.tile([128, nc.vector.BN_STATS_DIM], F32, tag="st")
                nc.vector.bn_stats(out=stats, in_=xt)
                mv = ln.tile([128, nc.vector.BN_AGGR_DIM], F32, tag="mv")
                nc.vector.bn_aggr(out=mv, in_=stats)
                rstd = ln.tile([128, 1], F32, tag="rs")
                nc.scalar.activation(out=rstd, in_=mv[:, 1:2],
                                     func=mybir.ActivationFunctionType.Sqrt,
                                     bias=eps_t, scale=1.0)
                nc.vector.reciprocal(out=rstd, in_=rstd)
                nmr = ln.tile([128, 1], F32, tag="nm")
                nc.vector.tensor_scalar(out=nmr, in0=mv[:, 0:1],
                                        scalar1=rstd, scalar2=-1.0,
                                        op0=mybir.AluOpType.mult,
                                        op1=mybir.AluOpType.mult)
                xn = ln.tile([128, D], F32, tag="xn")
                nc.scalar.activation(out=xn, in_=xt,
                                     func=mybir.ActivationFunctionType.Identity,
                                     bias=nmr[:, 0:1], scale=rstd[:, 0:1])
                nc.gpsimd.tensor_copy(zt_b[:, t, :], xn)
                for ct in range(CT):
                    pt = ps_ms.tile([128, 128], F32, tag="ms")
                    nc.tensor.transpose(pt, xn[:, ct * 128:(ct + 1) * 128], ident_f)
                    if ct < 2:
                        nc.vector.tensor_copy(zT[:, ct, t * 128:(t + 1) * 128], pt)
                    else:
                        nc.scalar.copy(zT[:, ct, t * 128:(t + 1) * 128], pt)
                    if t < 6:
                        warm(1)
                warm(2 if t < 6 else 1)
                if t == 3:
                    emit_proj(0, True)
                    emit_proj(0, False)
                    qmix0 = emit_qmix(0, 0)
                    pbs0, rs0 = open_block()
                elif t == 7:
                    emit_proj(1, True)
                    emit_proj(1, False)
                elif t == 11:
                    emit_proj(2, False)
                if t >= 4:
                    attn_step(qmix0, pbs0, rs0, t - 4)
            emit_proj(3, False)
            for jt in range(12, NT):
                attn_step(qmix0, pbs0, rs0, jt)
            close_block(0, 0, pbs0, rs0, wu0)

            # ---- remaining attention blocks ----
            for g in range(H):
                if g == 0:
                    wu_g = wu0
                else:
                    wu_g = gw.tile([128, CT, D], BF16, tag="wu")
                    nc.sync.dma_start(
                        out=wu_g,
                        in_=wu[:, :, g * D:(g + 1) * D].rearrange("t p c -> p t c"))
                for ch in range(NCH):
                    if g == 0 and ch == 0:
                        continue
                    qmix = emit_qmix(g, ch)
                    pbs, rs_acc = open_block()
                    for jt in range(NT):
                        attn_step(qmix, pbs, rs_acc, jt)
                    close_block(g, ch, pbs, rs_acc, wu_g)

            # ---- write out (bout already folded into the g==0 acc init) ----
            for it in range(QT):
                nc.sync.dma_start(out=out[it * 128:(it + 1) * 128, :],
                                  in_=acc[:, it, :])

    nc.compile()
    return nc


def prep_inputs(x, gamma, beta, Wq, Wkv, mix_pre, mix_post, Wout, bout):
    """Host-side weight fusion. Returns per-core in_maps."""
    import ml_dtypes
    x = np.asarray(x, np.float32)
    gamma = np.asarray(gamma, np.float32)
    beta = np.asarray(beta, np.float32)
    Wq = np.asarray(Wq, np.float32)
    Wkv = np.asarray(Wkv, np.float32)
    mix_pre = np.asarray(mix_pre, np.float32)
    mix_post = np.asarray(mix_post, np.float32)
    Wout = np.asarray(Wout, np.float32)
    bout = np.asarray(bout, np.float32)

    scale = DH ** -0.5
    Wk = Wkv[:, :D]
    Wv = Wkv[:, D:]

    qw = Wq * scale
    wqk_f = np.concatenate([gamma[:, None] * qw, gamma[:, None] * Wk], axis=1)
    bqk_f = np.concatenate([beta @ qw, beta @ Wk])
    # mixv[ct, p, g] = mix_pre[head of column ct*128+p, g]
    heads = (np.arange(D) // DH)
    mixv_f = mix_pre[heads, :].reshape(CT, 128, H)

    def fuse_u(wv_):  # (512, 8*512), col = h*512 + e
        return np.einsum("cgd,gde,hg->che", wv_.reshape(D, H, DH),
                         Wout.reshape(H, DH, D), mix_post).reshape(D, H * D)

    wqk_np = np.ascontiguousarray(
        wqk_f.reshape(CT, 128, 2 * D).astype(np.float16))
    bqk_np = np.ascontiguousarray(bqk_f.reshape(8, 128))
    mixv_np = np.ascontiguousarray(mixv_f)
    wu_np = np.ascontiguousarray(
        fuse_u(gamma[:, None] * Wv).reshape(CT, 128, H * D).astype(ml_dtypes.bfloat16))
    bo_np = np.ascontiguousarray(
        (bout + (beta @ fuse_u(Wv)).reshape(H, D).sum(0)).reshape(1, D))

    in_maps = []
    for c in range(N_CORES):
        b, half = divmod(c, 2)
        if half == 0:
            xb_c = x[b]
        else:  # query rows first; key order is irrelevant to the output
            xb_c = np.concatenate([x[b][NQ:], x[b][:NQ]], axis=0)
        in_maps.append({
            "xb": np.ascontiguousarray(xb_c),
            "wqk": wqk_np, "bqk": bqk_np, "mixv": mixv_np,
            "wu": wu_np, "bo": bo_np,
        })
    return in_maps


_NC_CACHE = {}


def get_program(attn_dtype=BF16):
    key = str(attn_dtype)
    if key not in _NC_CACHE:
        _NC_CACHE[key] = build_program(attn_dtype)
    return _NC_CACHE[key]


def run(in_maps, trace=False, **kw):
    nc = get_program()
    return run_bass_kernel_spmd(nc, in_maps, list(range(N_CORES)), trace=trace, **kw)


def kernel(x, gamma, beta, Wq, Wkv, mix_pre, mix_post, Wout, bout):
    in_maps = prep_inputs(x, gamma, beta, Wq, Wkv, mix_pre, mix_post, Wout, bout)
    res = run(in_maps)
    out = np.empty((B, N, D), np.float32)
    for c in range(N_CORES):
        b, half = divmod(c, 2)
        out[b, half * NQ:(half + 1) * NQ, :] = res.results[c]["out"]
    return out


# revision 24
# speedup vs baseline: 1.0175x; 1.0175x over previous
"""Talking-heads attention (b=4, n=2048, d=512, h=8) on 8 TRN2 NeuronCores.

Strategy
--------
dots2_g = sum_h mix_pre[h,g] (q_h . k_h) has the head-sum inside a 512-wide
contraction, so the dots matmuls contract (h,d)=512:

  attT_g[j,i] = sum_{hd} kT[hd,j] * (mix_pre[h,g] * qT[hd,i])

The g-dependence is only a per-64-row-block SCALE of qT, applied on the
Vector engine (qmix_g), so the per-g expanded-weight projections of the
old scheme disappear: one plain q/k projection (fp16) serves all heads,
and the dots stationary (kT) is head-independent.  mix_post + Wv + Wout +
gamma fold into expanded weights U (wu):

  B_g   = attn_g @ z          (z = normalized x, token-major, bf16)
  final = sum_g (B_g / rowsum_g) @ Wu_g + bout'

Rowsums run OFF the PE: exp tiles are accumulated over key-tiles on the
Vector engine and one f32r ones-matmul per (g, query-chunk) does the
partition reduction.  1/rowsum folds into the B PSUM evacuation.

Sharding: core c handles batch c//2 and query-row half c%2 (1024 rows) — no
collectives.  Each core's batch rows are permuted host-side so its query
rows come first (attention is invariant to key order).  Softmax runs
without max-subtraction (logits bounded ~25; exp fits bf16).
"""

import os
import sys

import numpy as np

for _p in ("/opt/trn_rl_repo", "/root/.axon_site/_ro/trn_rl_repo"):
    if os.path.isdir(_p) and _p not in sys.path:
        sys.path.append(_p)

import concourse.bacc as bacc
import concourse.tile as tile
from concourse import mybir
from concourse import masks
from concourse.bass_utils import run_bass_kernel_spmd

F32 = mybir.dt.float32
F32R = mybir.dt.float32r
BF16 = mybir.dt.bfloat16
FP16 = mybir.dt.float16

B, N, D = 4, 2048, 512
H, DH = 8, 64
NQ = N // 2            # query rows per core
N_CORES = 8
EPS = 1e-5
CT = D // 128           # 4 contraction tiles
NT = N // 128           # 16 token tiles (full sequence)
QT = NQ // 128          # 8 query tiles per core
NCH = NQ // 512         # 2 query chunks of 512


def build_program(attn_dtype=BF16):
    """Build + compile the SPMD single-core program. Returns nc."""
    nc = bacc.Bacc(
        "TRN2",
        target_bir_lowering=False,
        debug=False,
        enable_asserts=True,
        num_devices=1,
    )

    xb = nc.dram_tensor("xb", [N, D], F32, kind="ExternalInput").ap()
    wqk = nc.dram_tensor("wqk", [CT, 128, 2 * D], FP16, kind="ExternalInput").ap()
    bqk = nc.dram_tensor("bqk", [8, 128], F32, kind="ExternalInput").ap()
    mixv = nc.dram_tensor("mixv", [CT, 128, H], F32, kind="ExternalInput").ap()
    wu = nc.dram_tensor("wu", [CT, 128, H * D], BF16, kind="ExternalInput").ap()
    bo = nc.dram_tensor("bo", [1, D], F32, kind="ExternalInput").ap()
    out = nc.dram_tensor("out", [NQ, D], F32, kind="ExternalOutput").ap()

    import concourse.bass as bass

    with tile.TileContext(nc) as tc:
        with (
            tc.tile_pool(name="const", bufs=1) as const,
            tc.tile_pool(name="persist", bufs=1) as persist,
            tc.tile_pool(name="gw", bufs=2) as gw,
            tc.tile_pool(name="qm", bufs=2) as qm,
            tc.tile_pool(name="ln", bufs=6) as ln,
            tc.tile_pool(name="attw", bufs=12) as attw_pool,
            tc.tile_pool(name="rsp", bufs=2) as rsp,
            tc.tile_pool(name="btp", bufs=2) as btp,
            tc.tile_pool(name="rin", bufs=2) as rin,
            tc.tile_pool(name="ps_pd", bufs=2, space="PSUM") as ps_pd,
            tc.tile_pool(name="ps_pb", bufs=4, space="PSUM") as ps_pb,
            tc.tile_pool(name="ps_ms", bufs=2, space="PSUM") as ps_ms,
        ):
            ident_f = const.tile([128, 128], F32)
            masks.make_identity(nc, ident_f)
            ones_f = const.tile([128, 128], F32)
            nc.vector.memset(ones_f, 1.0)
            ones_r = const.tile([128, 128], F32R)
            nc.vector.tensor_copy(ones_r, ones_f)
            warm_b = const.tile([128, 256], BF16)
            nc.vector.memset(warm_b, 0.0)
            warm_s = const.tile([128, 128], BF16)
            nc.vector.memset(warm_s, 0.0)

            def warm(n):
                # keep the PE HAM clock-gate warm through DVE-bound stretches
                for _ in range(n):
                    pw = ps_pd.tile([128, 256], F32, tag="pd", name="pw")
                    nc.tensor.matmul(pw, warm_s, warm_b, start=True, stop=True)
            eps_t = const.tile([128, 1], F32)
            nc.vector.memset(eps_t, EPS)

            wqk_sb = const.tile([128, CT, 2 * D], FP16)
            bqk_sb = const.tile([128, 8], F32)
            mixv_sb = const.tile([128, CT, H], F32)
            bo_sb = const.tile([128, D], F32)

            zT = persist.tile([128, CT, N], FP16)     # normalized x, feature-major
            zt_b = persist.tile([128, NT, D], BF16)   # normalized x, token-major bf16
            qT = persist.tile([128, CT, NQ], FP16)    # q^T (scale folded), head-major
            kT = persist.tile([128, CT, N], FP16)     # k^T, head-major
            acc = persist.tile([128, QT, D], F32)     # final accumulator

            # ---- helpers ----
            def emit_proj(ic, qside):
                for co in (range(0, 4) if qside else range(4, 8)):
                    p = ps_ms.tile([128, 512], F32, tag="ms", name="pj")
                    for ct in range(CT):
                        nc.tensor.matmul(
                            p,
                            wqk_sb[:, ct, co * 128:(co + 1) * 128],
                            zT[:, ct, ic * 512:(ic + 1) * 512],
                            start=(ct == 0), stop=(ct == CT - 1),
                        )
                    dst = (qT[:, co, ic * 512:(ic + 1) * 512] if qside
                           else kT[:, co - 4, ic * 512:(ic + 1) * 512])
                    nc.scalar.activation(out=dst, in_=p,
                                         func=mybir.ActivationFunctionType.Identity,
                                         bias=bqk_sb[:, co:co + 1], scale=1.0)

            def emit_qmix(g, ch):
                # qmix = mix_pre[h,g] * qT chunk (per-64-row-block scale, DVE)
                qmix = qm.tile([128, CT, 512], FP16, tag="qx", name="qmix")
                for ct in range(CT):
                    nc.vector.tensor_scalar_mul(
                        qmix[:, ct, :], qT[:, ct, ch * 512:(ch + 1) * 512],
                        mixv_sb[:, ct, g:g + 1])
                return qmix

            def open_block():
                pbs = [ps_pb.tile([128, 512], F32, tag="pb", name=f"pb{cb}")
                       for cb in range(CT)]
                rs_acc = rsp.tile([128, 512], F32R, tag="rs")
                return pbs, rs_acc

            def dots_half(qmix, rs_acc, jt):
                pd = ps_pd.tile([128, 512], F32, tag="pd")
                for ct in range(CT):
                    nc.tensor.matmul(
                        pd, kT[:, ct, jt * 128:(jt + 1) * 128], qmix[:, ct, :],
                        start=(ct == 0), stop=(ct == CT - 1))
                aw = attw_pool.tile([128, 512], attn_dtype, tag="aw")
                nc.scalar.activation(out=aw, in_=pd,
                                     func=mybir.ActivationFunctionType.Exp)
                if jt == 0:
                    nc.vector.tensor_copy(rs_acc, aw)
                else:
                    nc.vector.tensor_add(out=rs_acc, in0=rs_acc, in1=aw)
                return aw

            def b_half(pbs, aw, jt):
                for cb in range(CT):
                    nc.tensor.matmul(pbs[cb], zt_b[:, jt, cb * 128:(cb + 1) * 128],
                                     aw, start=(jt == 0), stop=(jt == NT - 1))

            def attn_step(qmix, pbs, rs_acc, jt):
                aw = dots_half(qmix, rs_acc, jt)
                b_half(pbs, aw, jt)

            def close_block(g, ch, pbs, rs_acc, wu_g):
                # partition-reduce rowsums with one f32r matmul; 1/r folds into
                # the B evacuation; bout folds into the g==0 acc init
                prs = ps_ms.tile([128, 512], F32, tag="ms", name="prs")
                nc.tensor.matmul(prs, ones_r, rs_acc, start=True, stop=True)
                rinv = rin.tile([128, 512], F32, tag="ri")
                nc.vector.reciprocal_approx_fast(rinv, prs)
                bt = btp.tile([128, CT, 512], attn_dtype, tag="bt")
                for cb in range(CT):
                    nc.vector.tensor_mul(out=bt[:, cb, :], in0=pbs[cb], in1=rinv)
                for io in range(4):
                    it = ch * 4 + io
                    pf = ps_ms.tile([128, D], F32, tag="ms", name="pf")
                    for cb in range(CT):
                        nc.tensor.matmul(pf, bt[:, cb, io * 128:(io + 1) * 128],
                                         wu_g[:, cb, :],
                                         start=(cb == 0), stop=(cb == CT - 1))
                    nc.vector.tensor_add(out=acc[:, it, :], in0=pf,
                                         in1=(bo_sb if g == 0 else acc[:, it, :]))

            # ---- Phase 1+2 fused with the first attention block (g=0,ch=0):
            # LayerNorm tile t feeds transposes; projections are emitted as
            # their zT chunks complete; block0 attention steps trail t by 4.
            warm(16)
            xts = []
            for t in range(4):
                xt = ln.tile([128, D], F32, tag="xt", name=f"xt{t}")
                nc.sync.dma_start(out=xt, in_=xb[t * 128:(t + 1) * 128, :])
                xts.append(xt)
            nc.sync.dma_start(out=wqk_sb, in_=wqk.rearrange("t p c -> p t c"))
            nc.sync.dma_start(out=bqk_sb, in_=bqk.rearrange("t p -> p t"))
            nc.sync.dma_start(out=mixv_sb, in_=mixv.rearrange("t p c -> p t c"))
            nc.sync.dma_start(
                out=bo_sb,
                in_=bass.AP(tensor=bo.tensor, offset=bo.offset,
                            ap=[[0, 128], bo.ap[1]]),
            )
            wu0 = gw.tile([128, CT, D], BF16, tag="wu", name="wu0")
            nc.sync.dma_start(out=wu0, in_=wu[:, :, 0:D].rearrange("t p c -> p t c"))

            qmix0 = pbs0 = rs0 = None
            for t in range(NT):
                if t < 4:
                    xt = xts[t]
                else:
                    xt = ln.tile([128, D], F32, tag="xt")
                    nc.sync.dma_start(out=xt, in_=xb[t * 128:(t + 1) * 128, :])
                stats = ln.tile([128, nc.vector.BN_STATS_DIM], F32, tag="st")
                nc.vector.bn_stats(out=stats, in_=xt)
                mv = ln.tile([128, nc.vector.BN_AGGR_DIM], F32, tag="mv")
                nc.vector.bn_aggr(out=mv, in_=stats)
                rstd = ln.tile([128, 1], F32, tag="rs")
                nc.scalar.activation(out=rstd, in_=mv[:, 1:2],
                                     func=mybir.ActivationFunctionType.Sqrt,
                                     bias=eps_t, scale=1.0)
                nc.vector.reciprocal(out=rstd, in_=rstd)
                nmr = ln.tile([128, 1], F32, tag="nm")
                nc.vector.tensor_scalar(out=nmr, in0=mv[:, 0:1],
                                        scalar1=rstd, scalar2=-1.0,
                                        op0=mybir.AluOpType.mult,
                                        op1=mybir.AluOpType.mult)
                xn = ln.tile([128, D], F32, tag="xn")
                nc.scalar.activation(out=xn, in_=xt,
                                     func=mybir.ActivationFunctionType.Identity,
                                     bias=nmr[:, 0:1], scale=rstd[:, 0:1])
                nc.gpsimd.tensor_copy(zt_b[:, t, :], xn)
                for ct in range(CT):
                    pt = ps_ms.tile([128, 128], F32, tag="ms")
                    nc.tensor.transpose(pt, xn[:, ct * 128:(ct + 1) * 128], ident_f)
                    if ct < 2:
                        nc.vector.tensor_copy(zT[:, ct, t * 128:(t + 1) * 128], pt)
                    else:
                        nc.scalar.copy(zT[:, ct, t * 128:(t + 1) * 128], pt)
                    if t < 6:
                        warm(1)
                warm(2 if t < 6 else 1)
                if t == 3:
                    emit_proj(0, True)
                    emit_proj(0, False)
                    qmix0 = emit_qmix(0, 0)
                    pbs0, rs0 = open_block()
                elif t == 7:
                    emit_proj(1, True)
                    emit_proj(1, False)
                elif t == 11:
                    emit_proj(2, False)
                if t >= 4:
                    attn_step(qmix0, pbs0, rs0, t - 4)
            emit_proj(3, False)
            for jt in range(12, NT):
                attn_step(qmix0, pbs0, rs0, jt)
            close_block(0, 0, pbs0, rs0, wu0)

            # ---- remaining attention blocks ----
            for g in range(H):
                if g == 0:
                    wu_g = wu0
                else:
                    wu_g = gw.tile([128, CT, D], BF16, tag="wu")
                    nc.sync.dma_start(
                        out=wu_g,
                        in_=wu[:, :, g * D:(g + 1) * D].rearrange("t p c -> p t c"))
                for ch in range(NCH):
                    if g == 0 and ch == 0:
                        continue
                    qmix = emit_qmix(g, ch)
                    pbs, rs_acc = open_block()
                    # software-pipelined emission: dots(jt+1) ahead of B(jt) so
                    # the in-order PE queue never waits on the exp of jt
                    aw_prev = dots_half(qmix, rs_acc, 0)
                    for jt in range(1, NT):
                        aw = dots_half(qmix, rs_acc, jt)
                        b_half(pbs, aw_prev, jt - 1)
                        aw_prev = aw
                    b_half(pbs, aw_prev, NT - 1)
                    close_block(g, ch, pbs, rs_acc, wu_g)
                    if g == H - 1:  # stream results out as they complete
                        for io in range(4):
                            it = ch * 4 + io
                            nc.sync.dma_start(
                                out=out[it * 128:(it + 1) * 128, :],
                                in_=acc[:, it, :])

    nc.compile()
    return nc


def prep_inputs(x, gamma, beta, Wq, Wkv, mix_pre, mix_post, Wout, bout):
    """Host-side weight fusion. Returns per-core in_maps."""
    import ml_dtypes
    x = np.asarray(x, np.float32)
    gamma = np.asarray(gamma, np.float32)
    beta = np.asarray(beta, np.float32)
    Wq = np.asarray(Wq, np.float32)
    Wkv = np.asarray(Wkv, np.float32)
    mix_pre = np.asarray(mix_pre, np.float32)
    mix_post = np.asarray(mix_post, np.float32)
    Wout = np.asarray(Wout, np.float32)
    bout = np.asarray(bout, np.float32)

    scale = DH ** -0.5
    Wk = Wkv[:, :D]
    Wv = Wkv[:, D:]

    qw = Wq * scale
    wqk_f = np.concatenate([gamma[:, None] * qw, gamma[:, None] * Wk], axis=1)
    bqk_f = np.concatenate([beta @ qw, beta @ Wk])
    # mixv[ct, p, g] = mix_pre[head of column ct*128+p, g]
    heads = (np.arange(D) // DH)
    mixv_f = mix_pre[heads, :].reshape(CT, 128, H)

    def fuse_u(wv_):  # (512, 8*512), col = h*512 + e
        return np.einsum("cgd,gde,hg->che", wv_.reshape(D, H, DH),
                         Wout.reshape(H, DH, D), mix_post).reshape(D, H * D)

    wqk_np = np.ascontiguousarray(
        wqk_f.reshape(CT, 128, 2 * D).astype(np.float16))
    bqk_np = np.ascontiguousarray(bqk_f.reshape(8, 128))
    mixv_np = np.ascontiguousarray(mixv_f)
    wu_np = np.ascontiguousarray(
        fuse_u(gamma[:, None] * Wv).reshape(CT, 128, H * D).astype(ml_dtypes.bfloat16))
    bo_np = np.ascontiguousarray(
        (bout + (beta @ fuse_u(Wv)).reshape(H, D).sum(0)).reshape(1, D))

    in_maps = []
    for c in range(N_CORES):
        b, half = divmod(c, 2)
        if half == 0:
            xb_c = x[b]
        else:  # query rows first; key order is irrelevant to the output
            xb_c = np.concatenate([x[b][NQ:], x[b][:NQ]], axis=0)
        in_maps.append({
            "xb": np.ascontiguousarray(xb_c),
            "wqk": wqk_np, "bqk": bqk_np, "mixv": mixv_np,
            "wu": wu_np, "bo": bo_np,
        })
    return in_maps


_NC_CACHE = {}


def get_program(attn_dtype=BF16):
    key = str(attn_dtype)
    if key not in _NC_CACHE:
        _NC_CACHE[key] = build_program(attn_dtype)
    return _NC_CACHE[key]


def run(in_maps, trace=False, **kw):
    nc = get_program()
    return run_bass_kernel_spmd(nc, in_maps, list(range(N_CORES)), trace=trace, **kw)


def kernel(x, gamma, beta, Wq, Wkv, mix_pre, mix_post, Wout, bout):
    in_maps = prep_inputs(x, gamma, beta, Wq, Wkv, mix_pre, mix_post, Wout, bout)
    res = run(in_maps)
    out = np.empty((B, N, D), np.float32)
    for c in range(N_CORES):
        b, half = divmod(c, 2)
        out[b, half * NQ:(half + 1) * NQ, :] = res.results[c]["out"]
    return out
